# revision 4
# baseline (speedup 1.0000x reference)
"""Multi-head attention block (B=4, N=2048, C=1024, H=16) on 8 trn2 cores.

Sharding: core c handles batch c//2 and heads (c%2)*8 .. (c%2)*8+8
(data parallel on B, tensor parallel on heads). Each core computes
qkv projections for its 8 heads, attention, and a partial output
projection (row-parallel over W_proj); the host sums the two partial
projections per batch and adds b_proj. The host also pre-transposes
x (ships xT) and pre-casts weights/activations to bf16 — pure data
layout/sharding prep.

Per-core dataflow (layouts chosen so no on-device transposes are
needed):
  qT/kT[hd, m] = Wqk.T @ x.T   (W-stationary, bf16, psum-accum over k)
  v[n, hd]     = x @ Wv        (xT-stationary, bf16)
  St[n, m]     = k @ q.T       (kT-stationary, bf16, 2-head row-packed:
                                the two heads' score matmuls run in
                                separate PE row-quadrants concurrently)
  E = exp(St/8)                (alternating: ScalarE native exp / DVE
                                1-pass Schraudolph: int16(a*s+b) written
                                into the bf16 tile via bitcast)
  av[d, m]     = v.T @ E       (bf16, 2-head col-packed, psum-accum over n)
  sums[m]      = ones64.T @ E  (replicated across 64 partitions by the
                                PE so no partition-broadcast is needed)
  att[d, m]    = av * approx_recip(sums)   (DVE)
  out_part     = att.T @ Wp    (bf16, psum-accum over head pairs,
                                DMA'd to DRAM straight from PSUM)

Schedule: the Scalar-engine exp stream (~295us if done alone) is the
co-bottleneck with the PE (~273us of matmul streaming), so exp tiles
alternate between ScalarE and DVE, PSUM-drain copies alternate between
ScalarE and DVE, and all non-attention PE work (qkv projections, v
production, output projection) is emitted as paced "filler" chunks
inside the attention n-loops so the PE never waits on exp.
"""

import numpy as np
import ml_dtypes

import concourse.bass as bass
import concourse.mybir as mybir
import concourse.tile as tile
from concourse import bacc
from concourse.bass_utils import run_bass_kernel_spmd

F32 = mybir.dt.float32
BF16 = mybir.dt.bfloat16
I16 = mybir.dt.int16
EXP = mybir.ActivationFunctionType.Exp
MUL = mybir.AluOpType.mult
ADD = mybir.AluOpType.add

N = 2048          # sequence length
C = 1024          # model dim
DH = 64           # head dim
HPC = 8           # heads per core
P = 128           # partitions
NT = N // P       # 16 n/m tiles
KT = C // P       # 8 contraction tiles for qkv
MC = N // 512     # 4 m-chunks of 512
PAIRS = HPC // 2  # 4 head pairs
SCALE = 1.0 / np.sqrt(DH)

# Schraudolph exp on DVE: exp(s*SCALE) ~ bitcast_bf16(int16(s*A + B)).
# C_OFF tuned on the score distribution for min rms relative error.
C_OFF = -7.5
A_SCH = float(SCALE * 128.0 / np.log(2.0))
B_SCH = float(127.0 * 128.0 + C_OFF)


def _emit(nc, tc, ctx):
    xT_d = nc.dram_tensor("xT", [C, N], BF16, kind="ExternalInput").ap()
    # wqk: per pair p, cols [p*2048, (p+1)*2048) = 8 q k-tiles then 8 k k-tiles
    wqk_d = nc.dram_tensor("wqk", [P, PAIRS * 2048], BF16, kind="ExternalInput").ap()
    wv_d = nc.dram_tensor("wv", [P, KT * 512], BF16, kind="ExternalInput").ap()
    wp_d = nc.dram_tensor("wp", [P, PAIRS * C], BF16, kind="ExternalInput").ap()
    out_d = nc.dram_tensor("out", [N, C], F32, kind="ExternalOutput").ap()

    # --- pools ---
    consts = ctx.enter_context(tc.tile_pool(name="consts", bufs=1))
    sb_xT = ctx.enter_context(tc.tile_pool(name="sb_xT", bufs=KT))
    sb_wqk = ctx.enter_context(tc.tile_pool(name="sb_wqk", bufs=2))
    sb_wvp = ctx.enter_context(tc.tile_pool(name="sb_wvp", bufs=2))
    sb_v = ctx.enter_context(tc.tile_pool(name="sb_v", bufs=NT))
    sb_qkT = ctx.enter_context(tc.tile_pool(name="sb_qkT", bufs=4))
    sb_se = ctx.enter_context(tc.tile_pool(name="sb_se", bufs=6))
    sb_rc = ctx.enter_context(tc.tile_pool(name="sb_rc", bufs=2))
    sb_att = ctx.enter_context(tc.tile_pool(name="sb_att", bufs=PAIRS * MC))
    sb_out = ctx.enter_context(tc.tile_pool(name="sb_out", bufs=3))

    ps_sc = ctx.enter_context(tc.tile_pool(name="ps_sc", bufs=2, space="PSUM"))
    ps_av = ctx.enter_context(tc.tile_pool(name="ps_av", bufs=1, space="PSUM"))
    ps_sm = ctx.enter_context(tc.tile_pool(name="ps_sm", bufs=1, space="PSUM"))
    ps_small = ctx.enter_context(tc.tile_pool(name="ps_small", bufs=2, space="PSUM"))

    # --- constants ---
    ones_bf = consts.tile([P, DH], BF16)
    nc.vector.memset(ones_bf, 1.0)

    # --- aux copy round-robin between ScalarE (activation Copy) and DVE ---
    aux = {"t": 0}

    def aux_copy(dst, src):
        if aux["t"] & 1:
            nc.scalar.copy(dst, src)
        else:
            nc.vector.tensor_copy(dst, src)
        aux["t"] += 1

    # --- input DMAs: pair-0 weights first, then xT in mc-major chunks ---
    wqk_sb = [None] * PAIRS

    def load_wqk(p):
        w = sb_wqk.tile([P, 2048], BF16, tag="wqk", name=f"wqk{p}")
        nc.sync.dma_start(out=w, in_=wqk_d[:, p * 2048:(p + 1) * 2048])
        wqk_sb[p] = w

    load_wqk(0)
    xT = [sb_xT.tile([P, N], BF16, tag="xT", name=f"xT{k}") for k in range(KT)]
    for mc2 in range(2):
        for k in range(KT):
            nc.sync.dma_start(
                out=xT[k][:, mc2 * 1024:(mc2 + 1) * 1024],
                in_=xT_d[k * P:(k + 1) * P, mc2 * 1024:(mc2 + 1) * 1024],
            )
    wv_t = sb_wvp.tile([P, KT * 512], BF16, tag="wvp", name="wv")
    nc.sync.dma_start(out=wv_t, in_=wv_d)
    wp_t = sb_wvp.tile([P, PAIRS * C], BF16, tag="wvp", name="wp")
    nc.sync.dma_start(out=wp_t, in_=wp_d)

    # --- qkT production for pair p, as 8 filler chunks (half, mc4) ---
    def qkT_chunk(p, half, mc4, dst):
        ps = ps_small.tile([P, 512], F32, tag="ps_small", name=f"qkps{p}_{half}_{mc4}")
        for k in range(KT):
            nc.tensor.matmul(
                ps,
                wqk_sb[p][:, half * 1024 + k * P:half * 1024 + (k + 1) * P],
                xT[k][:, mc4 * 512:(mc4 + 1) * 512],
                start=(k == 0), stop=(k == KT - 1),
            )
        aux_copy(dst[:, mc4 * 512:(mc4 + 1) * 512], ps)

    def qkT_fillers(p):
        qT = sb_qkT.tile([P, N], BF16, tag="qkT", name=f"qT{p}")
        kT = sb_qkT.tile([P, N], BF16, tag="qkT", name=f"kT{p}")
        fillers = []
        for half, dst in ((0, qT), (1, kT)):
            for mc4 in range(MC):
                fillers.append(
                    lambda p=p, half=half, mc4=mc4, dst=dst:
                        qkT_chunk(p, half, mc4, dst))
        return (qT, kT), fillers

    # --- v production: half 0 = head cols 0:256 (pairs 0-1), half 1 = 256:512 ---
    v_sb = [None] * NT

    def v_chunk(m, half):
        if v_sb[m] is None:
            v_sb[m] = sb_v.tile([P, HPC * DH], BF16, tag="v", name=f"v{m}")
        ps = ps_small.tile([P, 256], F32, tag="ps_small", name=f"vps{m}_{half}")
        for k in range(KT):
            nc.tensor.matmul(
                ps, xT[k][:, m * P:(m + 1) * P],
                wv_t[:, k * 512 + half * 256:k * 512 + half * 256 + 256],
                start=(k == 0), stop=(k == KT - 1),
            )
        aux_copy(v_sb[m][:, half * 256:half * 256 + 256], ps)

    # --- output projection for one m-tile, DMA'd straight from PSUM ---
    att_tiles = {}

    def proj_chunk(mc, m4):
        m = mc * 4 + m4
        ot = sb_out.tile([P, C], F32, tag="out", name=f"out{m}")
        for cc in range(2):
            ps = ps_small.tile([P, 512], F32, tag="ps_small", name=f"pps{m}_{cc}")
            for p in range(PAIRS):
                nc.tensor.matmul(
                    ps,
                    att_tiles[(p, mc)][:, m4 * P:(m4 + 1) * P],
                    wp_t[:, p * C + cc * 512:p * C + (cc + 1) * 512],
                    start=(p == 0), stop=(p == PAIRS - 1),
                )
            aux_copy(ot[:, cc * 512:(cc + 1) * 512], ps)
        nc.sync.dma_start(out=out_d[m * P:(m + 1) * P, :], in_=ot)

    # --- attention phase (p, mc): 16 n-tiles with paced fillers ---
    exp_tog = {"t": 0}

    def emit_attention(p, qT, kT, mc, fillers=(), v_hook=False):
        av = ps_av.tile([P, 512], F32, tag="av", name=f"av{p}_{mc}")
        sm = ps_sm.tile([P, 512], F32, tag="sm", name=f"sm{p}_{mc}")
        LAG = 2  # av/sums trail scores/exp to hide the exp->av sem latency
        ses = {}
        # spread fillers evenly across the 16 n slots
        slots = {}
        nf = len(fillers)
        for j in range(nf):
            slots.setdefault(((j + 1) * NT) // (nf + 1), []).append(fillers[j])

        def emit_avsm(n):
            se = ses.pop(n)
            first, last = (n == 0), (n == NT - 1)
            for h in range(2):
                hd = p * P + h * DH
                nc.tensor.matmul(
                    av[h * DH:(h + 1) * DH, :],
                    v_sb[n][:, hd:hd + DH],
                    se[:, h * 512:(h + 1) * 512],
                    start=first, stop=last, skip_group_check=True,
                )
            for h in range(2):
                nc.tensor.matmul(
                    sm[h * DH:(h + 1) * DH, :],
                    ones_bf,
                    se[:, h * 512:(h + 1) * 512],
                    start=first, stop=last, skip_group_check=True,
                )

        for n in range(NT):
            sc = ps_sc.tile([P, 1024], F32, tag="sc", name=f"sc{p}_{mc}_{n}")
            for h in range(2):
                lo, hi = h * DH, (h + 1) * DH
                nc.tensor.matmul(
                    sc[:, h * 512:(h + 1) * 512],
                    kT[lo:hi, n * P:(n + 1) * P],
                    qT[lo:hi, mc * 512:(mc + 1) * 512],
                    start=True, stop=True, skip_group_check=True,
                )
            se = sb_se.tile([P, 1024], BF16, tag="se", name=f"se{p}_{mc}_{n}")
            if exp_tog["t"] & 1:
                nc.vector.tensor_scalar(
                    out=se.bitcast(I16), in0=sc,
                    scalar1=A_SCH, scalar2=B_SCH, op0=MUL, op1=ADD)
            else:
                nc.scalar.activation(se, sc, EXP, scale=float(SCALE))
            exp_tog["t"] += 1
            ses[n] = se
            if v_hook:
                v_chunk(n, 0)
            for f in slots.get(n, ()):
                f()
            if n >= LAG:
                emit_avsm(n - LAG)
        for n in range(NT - LAG, NT):
            emit_avsm(n)
        # normalize: att = av * (1/sums); sums are PE-replicated across
        # all 64 partitions per head, so no partition broadcast is needed.
        rc = sb_rc.tile([P, 512], F32, tag="rc", name=f"rc{p}_{mc}")
        nc.vector.reciprocal_approx_fast(rc, sm)
        att = sb_att.tile([P, 512], BF16, tag="att", name=f"att{p}_{mc}")
        nc.vector.tensor_tensor(att, av, rc, op=MUL)
        att_tiles[(p, mc)] = att

    # --- schedule ---
    # prologue: qkT(0) in DMA-arrival order (mc-major, q/k interleaved)
    (qT0, kT0), f0 = qkT_fillers(0)
    for mc4 in range(MC):
        for half in range(2):
            f0[half * MC + mc4]()
    qkT_cur = (qT0, kT0)

    vB = [lambda m=m: v_chunk(m, 1) for m in range(NT)]
    for p in range(PAIRS):
        qkT_next = None
        nxt = []
        if p + 1 < PAIRS:
            load_wqk(p + 1)
            qkT_next, nxt = qkT_fillers(p + 1)
        for mc in range(MC):
            fillers = []
            if p == 0 and mc > 0:
                fillers += vB[(mc - 1) * 6:mc * 6 if mc < 3 else NT]
                fillers += nxt[(mc - 1) * 3:mc * 3 if mc < 3 else 8]
            elif p > 0 and mc < 3:
                fillers += nxt[mc * 3:(mc + 1) * 3 if mc < 2 else 8]
            if p == 3 and mc > 0:
                fillers += [lambda mc=mc, m4=m4: proj_chunk(mc - 1, m4)
                            for m4 in range(4)]
            emit_attention(p, qkT_cur[0], qkT_cur[1], mc,
                           fillers=fillers, v_hook=(p == 0 and mc == 0))
        if qkT_next is not None:
            qkT_cur = qkT_next
    # epilogue: last output projection chunk
    for m4 in range(4):
        proj_chunk(3, m4)


def build_nc():
    from contextlib import ExitStack

    nc = bacc.Bacc("TRN2", target_bir_lowering=False, debug=False, num_devices=8)
    with tile.TileContext(nc) as tc:
        with ExitStack() as ctx:
            _emit(nc, tc, ctx)
    nc.compile()
    return nc


_NC = None


def _in_maps(x, W_qkv, W_proj):
    bf = ml_dtypes.bfloat16
    in_maps = []
    for c in range(8):
        b, h0 = c // 2, (c % 2) * HPC * DH  # h0 = col offset (0 or 512)
        # wqk: per pair p: 8 k-tiles of the q block then 8 of the k block
        wqk_blocks = []
        for p in range(PAIRS):
            for off in (h0 + p * P, C + h0 + p * P):
                for k in range(KT):
                    wqk_blocks.append(W_qkv[k * P:(k + 1) * P, off:off + P])
        wv_blocks = [W_qkv[k * P:(k + 1) * P, 2 * C + h0:2 * C + h0 + 512]
                     for k in range(KT)]
        wp_blocks = [W_proj[h0 + p * P:h0 + (p + 1) * P, :] for p in range(PAIRS)]
        in_maps.append({
            "xT": np.ascontiguousarray(x[b].T).astype(bf),
            "wqk": np.ascontiguousarray(np.concatenate(wqk_blocks, axis=1)).astype(bf),
            "wv": np.ascontiguousarray(np.concatenate(wv_blocks, axis=1)).astype(bf),
            "wp": np.ascontiguousarray(np.concatenate(wp_blocks, axis=1)).astype(bf),
        })
    return in_maps


def kernel(x, W_qkv, b_qkv, W_proj, b_proj):
    global _NC
    assert np.all(b_qkv == 0.0), "kernel assumes zero qkv bias"
    x = np.asarray(x, np.float32)
    W_qkv = np.asarray(W_qkv, np.float32)
    W_proj = np.asarray(W_proj, np.float32)
    b_proj = np.asarray(b_proj, np.float32)
    if _NC is None:
        _NC = build_nc()
    res = run_bass_kernel_spmd(_NC, _in_maps(x, W_qkv, W_proj), list(range(8)))
    out = np.empty((4, N, C), np.float32)
    for b in range(4):
        out[b] = res.results[2 * b]["out"] + res.results[2 * b + 1]["out"] + b_proj
    return out


# revision 6
# speedup vs baseline: 1.1943x; 1.1943x over previous
"""Multi-head attention block (B=4, N=2048, C=1024, H=16) on 8 trn2 cores.

Sharding: core c handles batch c//2 and heads (c%2)*8 .. (c%2)*8+8
(data parallel on B, tensor parallel on heads). Each core computes
qkv projections for its 8 heads, attention, and a partial output
projection (row-parallel over W_proj); the host sums the two partial
projections per batch and adds b_proj. The host also pre-transposes
x (ships xT) and pre-casts weights/activations to bf16 — pure data
layout/sharding prep.

Per-core dataflow (layouts chosen so no on-device transposes are
needed):
  qT/kT[hd, m] = Wqk.T @ x.T   (W-stationary, bf16, psum-accum over k)
  v[n, hd]     = x @ Wv        (xT-stationary, bf16)
  St[n, m]     = k @ q.T       (kT-stationary, bf16, 2-head row-packed:
                                the two heads' score matmuls run in
                                separate PE row-quadrants concurrently)
  E = exp(St/8)                (alternating: ScalarE native exp / DVE
                                1-pass Schraudolph: int16(a*s+b) written
                                into the bf16 tile via bitcast)
  av[d, m]     = v.T @ E       (bf16, 2-head col-packed, psum-accum over n)
  sums[m]      = ones64.T @ E  (replicated across 64 partitions by the
                                PE so no partition-broadcast is needed)
  att[d, m]    = av * approx_recip(sums)   (DVE)
  out_part     = att.T @ Wp    (bf16, psum-accum over head pairs,
                                DMA'd to DRAM straight from PSUM)

Schedule: the Scalar-engine exp stream (~295us if done alone) is the
co-bottleneck with the PE (~273us of matmul streaming), so exp tiles
alternate between ScalarE and DVE, PSUM-drain copies alternate between
ScalarE and DVE, and all non-attention PE work (qkv projections, v
production, output projection) is emitted as paced "filler" chunks
inside the attention n-loops so the PE never waits on exp.
"""

import numpy as np
import ml_dtypes

import concourse.bass as bass
import concourse.mybir as mybir
import concourse.tile as tile
from concourse import bacc
from concourse.bass_utils import run_bass_kernel_spmd

F32 = mybir.dt.float32
BF16 = mybir.dt.bfloat16
I16 = mybir.dt.int16
EXP = mybir.ActivationFunctionType.Exp
MUL = mybir.AluOpType.mult
ADD = mybir.AluOpType.add

N = 2048          # sequence length
C = 1024          # model dim
DH = 64           # head dim
HPC = 8           # heads per core
P = 128           # partitions
NT = N // P       # 16 n/m tiles
KT = C // P       # 8 contraction tiles for qkv
MC = N // 512     # 4 m-chunks of 512
PAIRS = HPC // 2  # 4 head pairs
SCALE = 1.0 / np.sqrt(DH)

# Schraudolph exp on DVE: exp(s*SCALE) ~ bitcast_bf16(int16(s*A + B)).
# C_OFF tuned on the score distribution for min rms relative error.
C_OFF = -7.5
A_SCH = float(SCALE * 128.0 / np.log(2.0))
B_SCH = float(127.0 * 128.0 + C_OFF)


def _emit(nc, tc, ctx):
    xT_d = nc.dram_tensor("xT", [C, N], BF16, kind="ExternalInput").ap()
    # wqk: per pair p, cols [p*2048, (p+1)*2048) = 8 q k-tiles then 8 k k-tiles
    wqk_d = nc.dram_tensor("wqk", [P, PAIRS * 2048], BF16, kind="ExternalInput").ap()
    wv_d = nc.dram_tensor("wv", [P, KT * 512], BF16, kind="ExternalInput").ap()
    wp_d = nc.dram_tensor("wp", [P, PAIRS * C], BF16, kind="ExternalInput").ap()
    out_d = nc.dram_tensor("out", [N, C], F32, kind="ExternalOutput").ap()

    # --- pools ---
    consts = ctx.enter_context(tc.tile_pool(name="consts", bufs=1))
    sb_xT = ctx.enter_context(tc.tile_pool(name="sb_xT", bufs=KT))
    sb_wqk = ctx.enter_context(tc.tile_pool(name="sb_wqk", bufs=2))
    sb_wvp = ctx.enter_context(tc.tile_pool(name="sb_wvp", bufs=2))
    sb_v = ctx.enter_context(tc.tile_pool(name="sb_v", bufs=NT))
    sb_qkT = ctx.enter_context(tc.tile_pool(name="sb_qkT", bufs=4))
    sb_se = ctx.enter_context(tc.tile_pool(name="sb_se", bufs=6))
    sb_rc = ctx.enter_context(tc.tile_pool(name="sb_rc", bufs=2))
    sb_att = ctx.enter_context(tc.tile_pool(name="sb_att", bufs=PAIRS * MC))
    sb_out = ctx.enter_context(tc.tile_pool(name="sb_out", bufs=3))

    ps_sc = ctx.enter_context(tc.tile_pool(name="ps_sc", bufs=2, space="PSUM"))
    ps_av = ctx.enter_context(tc.tile_pool(name="ps_av", bufs=1, space="PSUM"))
    ps_sm = ctx.enter_context(tc.tile_pool(name="ps_sm", bufs=1, space="PSUM"))
    ps_small = ctx.enter_context(tc.tile_pool(name="ps_small", bufs=2, space="PSUM"))

    # --- constants ---
    ones_bf = consts.tile([P, DH], BF16)
    nc.vector.memset(ones_bf, 1.0)

    # --- aux copy round-robin between ScalarE (activation Copy) and DVE ---
    aux = {"t": 0}

    def aux_copy(dst, src):
        if aux["t"] & 1:
            nc.scalar.copy(dst, src)
        else:
            nc.vector.tensor_copy(dst, src)
        aux["t"] += 1

    # --- input DMAs: pair-0 weights first, then xT in mc-major chunks ---
    wqk_sb = [None] * PAIRS

    def load_wqk(p):
        w = sb_wqk.tile([P, 2048], BF16, tag="wqk", name=f"wqk{p}")
        nc.sync.dma_start(out=w, in_=wqk_d[:, p * 2048:(p + 1) * 2048])
        wqk_sb[p] = w

    load_wqk(0)
    xT = [sb_xT.tile([P, N], BF16, tag="xT", name=f"xT{k}") for k in range(KT)]
    for mc2 in range(2):
        for k in range(KT):
            nc.sync.dma_start(
                out=xT[k][:, mc2 * 1024:(mc2 + 1) * 1024],
                in_=xT_d[k * P:(k + 1) * P, mc2 * 1024:(mc2 + 1) * 1024],
            )
    wv_t = sb_wvp.tile([P, KT * 512], BF16, tag="wvp", name="wv")
    nc.sync.dma_start(out=wv_t, in_=wv_d)
    wp_t = sb_wvp.tile([P, PAIRS * C], BF16, tag="wvp", name="wp")
    nc.sync.dma_start(out=wp_t, in_=wp_d)

    # --- qkT production for pair p, as 8 filler chunks (half, mc4) ---
    def qkT_chunk(p, half, mc4, dst):
        ps = ps_small.tile([P, 512], F32, tag="ps_small", name=f"qkps{p}_{half}_{mc4}")
        for k in range(KT):
            nc.tensor.matmul(
                ps,
                wqk_sb[p][:, half * 1024 + k * P:half * 1024 + (k + 1) * P],
                xT[k][:, mc4 * 512:(mc4 + 1) * 512],
                start=(k == 0), stop=(k == KT - 1),
            )
        aux_copy(dst[:, mc4 * 512:(mc4 + 1) * 512], ps)

    def qkT_fillers(p):
        qT = sb_qkT.tile([P, N], BF16, tag="qkT", name=f"qT{p}")
        kT = sb_qkT.tile([P, N], BF16, tag="qkT", name=f"kT{p}")
        fillers = []
        for half, dst in ((0, qT), (1, kT)):
            for mc4 in range(MC):
                fillers.append(
                    lambda p=p, half=half, mc4=mc4, dst=dst:
                        qkT_chunk(p, half, mc4, dst))
        return (qT, kT), fillers

    # --- v production: half 0 = head cols 0:256 (pairs 0-1), half 1 = 256:512 ---
    v_sb = [None] * NT

    def v_chunk(m, half):
        if v_sb[m] is None:
            v_sb[m] = sb_v.tile([P, HPC * DH], BF16, tag="v", name=f"v{m}")
        ps = ps_small.tile([P, 256], F32, tag="ps_small", name=f"vps{m}_{half}")
        for k in range(KT):
            nc.tensor.matmul(
                ps, xT[k][:, m * P:(m + 1) * P],
                wv_t[:, k * 512 + half * 256:k * 512 + half * 256 + 256],
                start=(k == 0), stop=(k == KT - 1),
            )
        aux_copy(v_sb[m][:, half * 256:half * 256 + 256], ps)

    # --- output projection for one m-tile, DMA'd straight from PSUM ---
    att_tiles = {}

    def proj_chunk(mc, m4):
        m = mc * 4 + m4
        ot = sb_out.tile([P, C], F32, tag="out", name=f"out{m}")
        for cc in range(2):
            ps = ps_small.tile([P, 512], F32, tag="ps_small", name=f"pps{m}_{cc}")
            for p in range(PAIRS):
                nc.tensor.matmul(
                    ps,
                    att_tiles[(p, mc)][:, m4 * P:(m4 + 1) * P],
                    wp_t[:, p * C + cc * 512:p * C + (cc + 1) * 512],
                    start=(p == 0), stop=(p == PAIRS - 1),
                )
            aux_copy(ot[:, cc * 512:(cc + 1) * 512], ps)
        nc.sync.dma_start(out=out_d[m * P:(m + 1) * P, :], in_=ot)

    # --- attention phase (p, mc): 16 n-tiles with paced fillers ---
    exp_tog = {"t": 0}

    def emit_attention(p, qT, kT, mc, fillers=(), v_hook=False):
        av = ps_av.tile([P, 512], F32, tag="av", name=f"av{p}_{mc}")
        sm = ps_sm.tile([P, 512], F32, tag="sm", name=f"sm{p}_{mc}")
        LAG = 2  # av/sums trail scores/exp to hide the exp->av sem latency
        ses = {}
        # spread fillers evenly across the 16 n slots
        slots = {}
        nf = len(fillers)
        for j in range(nf):
            slots.setdefault(((j + 1) * NT) // (nf + 1), []).append(fillers[j])

        def emit_av2(ns):
            # batched: both tiles' av pairs back-to-back (same quadrant
            # config, same psum accumulation target -> no PE drain between),
            # then both sums pairs. 3 config switches per 2 n-tiles.
            for n in ns:
                se = ses[n]
                for h in range(2):
                    hd = p * P + h * DH
                    nc.tensor.matmul(
                        av[h * DH:(h + 1) * DH, :],
                        v_sb[n][:, hd:hd + DH],
                        se[:, h * 512:(h + 1) * 512],
                        start=(n == 0), stop=(n == NT - 1),
                        skip_group_check=True,
                    )
            for n in ns:
                se = ses.pop(n)
                for h in range(2):
                    nc.tensor.matmul(
                        sm[h * DH:(h + 1) * DH, :],
                        ones_bf,
                        se[:, h * 512:(h + 1) * 512],
                        start=(n == 0), stop=(n == NT - 1),
                        skip_group_check=True,
                    )

        for n0 in range(0, NT, 2):
            sc2 = []
            for n in (n0, n0 + 1):
                sc = ps_sc.tile([P, 1024], F32, tag="sc", name=f"sc{p}_{mc}_{n}")
                sc2.append(sc)
                for h in range(2):
                    lo, hi = h * DH, (h + 1) * DH
                    nc.tensor.matmul(
                        sc[:, h * 512:(h + 1) * 512],
                        kT[lo:hi, n * P:(n + 1) * P],
                        qT[lo:hi, mc * 512:(mc + 1) * 512],
                        start=True, stop=True, skip_group_check=True,
                    )
            for n, sc in zip((n0, n0 + 1), sc2):
                se = sb_se.tile([P, 1024], BF16, tag="se", name=f"se{p}_{mc}_{n}")
                if exp_tog["t"] & 1:
                    nc.scalar.activation(se, sc, EXP, scale=float(SCALE))
                else:
                    nc.vector.tensor_scalar(
                        out=se.bitcast(I16), in0=sc,
                        scalar1=A_SCH, scalar2=B_SCH, op0=MUL, op1=ADD)
                exp_tog["t"] += 1
                ses[n] = se
                if v_hook:
                    v_chunk(n, 0)
            for f in list(slots.get(n0, [])) + list(slots.get(n0 + 1, [])):
                f()
            if n0 >= LAG:
                emit_av2((n0 - 2, n0 - 1))
        emit_av2((NT - 2, NT - 1))
        # normalize: att = av * (1/sums); sums are PE-replicated across
        # all 64 partitions per head, so no partition broadcast is needed.
        rc = sb_rc.tile([P, 512], F32, tag="rc", name=f"rc{p}_{mc}")
        nc.vector.reciprocal_approx_fast(rc, sm)
        att = sb_att.tile([P, 512], BF16, tag="att", name=f"att{p}_{mc}")
        nc.vector.tensor_tensor(att, av, rc, op=MUL)
        att_tiles[(p, mc)] = att

    # --- schedule ---
    # prologue: qkT(0) in DMA-arrival order (mc-major, q/k interleaved)
    (qT0, kT0), f0 = qkT_fillers(0)
    for mc4 in range(MC):
        for half in range(2):
            f0[half * MC + mc4]()
    qkT_cur = (qT0, kT0)

    vB = [lambda m=m: v_chunk(m, 1) for m in range(NT)]
    for p in range(PAIRS):
        qkT_next = None
        nxt = []
        if p + 1 < PAIRS:
            load_wqk(p + 1)
            qkT_next, nxt = qkT_fillers(p + 1)
        for mc in range(MC):
            fillers = []
            if p == 0 and mc > 0:
                fillers += vB[(mc - 1) * 6:mc * 6 if mc < 3 else NT]
                fillers += nxt[(mc - 1) * 3:mc * 3 if mc < 3 else 8]
            elif p > 0 and mc < 3:
                fillers += nxt[mc * 3:(mc + 1) * 3 if mc < 2 else 8]
            if p == 3 and mc > 0:
                fillers += [lambda mc=mc, m4=m4: proj_chunk(mc - 1, m4)
                            for m4 in range(4)]
            emit_attention(p, qkT_cur[0], qkT_cur[1], mc,
                           fillers=fillers, v_hook=(p == 0 and mc == 0))
        if qkT_next is not None:
            qkT_cur = qkT_next
    # epilogue: last output projection chunk
    for m4 in range(4):
        proj_chunk(3, m4)


def build_nc():
    from contextlib import ExitStack

    nc = bacc.Bacc("TRN2", target_bir_lowering=False, debug=False, num_devices=8)
    with tile.TileContext(nc) as tc:
        with ExitStack() as ctx:
            _emit(nc, tc, ctx)
    nc.compile()
    return nc


_NC = None


def _in_maps(x, W_qkv, W_proj):
    bf = ml_dtypes.bfloat16
    in_maps = []
    for c in range(8):
        b, h0 = c // 2, (c % 2) * HPC * DH  # h0 = col offset (0 or 512)
        # wqk: per pair p: 8 k-tiles of the q block then 8 of the k block
        wqk_blocks = []
        for p in range(PAIRS):
            for off in (h0 + p * P, C + h0 + p * P):
                for k in range(KT):
                    wqk_blocks.append(W_qkv[k * P:(k + 1) * P, off:off + P])
        wv_blocks = [W_qkv[k * P:(k + 1) * P, 2 * C + h0:2 * C + h0 + 512]
                     for k in range(KT)]
        wp_blocks = [W_proj[h0 + p * P:h0 + (p + 1) * P, :] for p in range(PAIRS)]
        in_maps.append({
            "xT": np.ascontiguousarray(x[b].T).astype(bf),
            "wqk": np.ascontiguousarray(np.concatenate(wqk_blocks, axis=1)).astype(bf),
            "wv": np.ascontiguousarray(np.concatenate(wv_blocks, axis=1)).astype(bf),
            "wp": np.ascontiguousarray(np.concatenate(wp_blocks, axis=1)).astype(bf),
        })
    return in_maps


def kernel(x, W_qkv, b_qkv, W_proj, b_proj):
    global _NC
    assert np.all(b_qkv == 0.0), "kernel assumes zero qkv bias"
    x = np.asarray(x, np.float32)
    W_qkv = np.asarray(W_qkv, np.float32)
    W_proj = np.asarray(W_proj, np.float32)
    b_proj = np.asarray(b_proj, np.float32)
    if _NC is None:
        _NC = build_nc()
    res = run_bass_kernel_spmd(_NC, _in_maps(x, W_qkv, W_proj), list(range(8)))
    out = np.empty((4, N, C), np.float32)
    for b in range(4):
        out[b] = res.results[2 * b]["out"] + res.results[2 * b + 1]["out"] + b_proj
    return out
